# revision 1
# baseline (speedup 1.0000x reference)
"""MinGRU forward on 8 TRN2 NeuronCores.

Math (linear-space reformulation of the reference's log-space Heinsen scan):
    hg = x @ W_hg.T                       # [B,S,2D]
    hidden, gate = split(hg)
    z = sigmoid(gate)
    c = sigmoid(-gate)                    # = 1 - z = exp(-softplus(gate))
    g = max(hidden + 0.5, sigmoid(hidden))  # == where(h>=0, h+0.5, sigmoid(h)) exactly
    u = z * g
    h[t] = c[t] * h[t-1] + u[t]           # convex combination -> bounded, stable
    out = h

The recurrence maps directly onto the DVE `tensor_tensor_scan` instruction
(state = data0*state + data1 along the free dim, fp32 internal state).

Sharding: 8 cores = 4 batches x 2 feature-halves (512 features each).
No cross-core communication: the scan is per-feature independent.
Host pre-transposes x (-> xT [D,S]) and W (-> wT [D, 2*512]) so the kernel
needs no on-chip transposes; matmul uses fp32r (fp32 with 11-bit mantissa,
full-rate on the PE).  Inputs are pre-rounded to fp32r on the host (RNE).
"""

import numpy as np

B, S, D = 4, 4096, 1024
DH = D // 2          # features per core
N_CORES = 8
SC = 512             # tokens per seq chunk (PSUM bank = 512 fp32)
NSC = S // SC        # 8 seq chunks
KC = 128             # contraction chunk
NKC = D // KC        # 8 k chunks
FC = 128             # feature chunk (psum partitions)
NFC = DH // FC       # 4 feature chunks

_CACHE = {}

# build-time knobs (A/B tuning; defaults are the shipped config)
CONFIG = {
    "xbufs": 2,
    "psbufs": 4,
    "ebufs": 3,
    "xsplit": False,   # split x chunk DMAs per k-chunk
    "u_on_gpsimd": False,  # compute u = z*g on GpSimd instead of DVE
    "mm_interleave": False,  # alternate h/g matmuls per k-chunk
    "split_last_scan": True,  # last chunk: 2 chained half-scans so out-DMA overlaps
}


def _round_fp32r(a: np.ndarray) -> np.ndarray:
    """Round fp32 array to fp32r (11 explicit mantissa bits) with RNE."""
    u = np.ascontiguousarray(a, dtype=np.float32).view(np.uint32)
    r = (u + np.uint32(0x7FF) + ((u >> np.uint32(12)) & np.uint32(1))) & np.uint32(0xFFFFF000)
    return r.view(np.float32)


def _build():
    import concourse.bacc as bacc
    import concourse.tile as tile
    import concourse.mybir as mybir

    f32 = mybir.dt.float32
    f32r = mybir.dt.float32r
    AF = mybir.ActivationFunctionType
    OP = mybir.AluOpType

    nc = bacc.Bacc("TRN2")
    xT = nc.dram_tensor("xT", [D, S], f32r, kind="ExternalInput")
    # wT layout: [D, NFC, 2*FC] — per feature-chunk fc, 128 hidden cols then
    # 128 gate cols, contiguous, so each fc's weights are one 1 MiB DMA.
    wT = nc.dram_tensor("wT", [D, NFC, 2 * FC], f32r, kind="ExternalInput")
    outT = nc.dram_tensor("outT", [DH, S], f32, kind="ExternalOutput")

    with tile.TileContext(nc) as tc:
        with (
            tc.tile_pool(name="w", bufs=1) as wpool,
            tc.tile_pool(name="x", bufs=CONFIG["xbufs"]) as xpool,
            tc.tile_pool(name="ew", bufs=CONFIG["ebufs"]) as epool,
            tc.tile_pool(name="h", bufs=2) as hpool,
            tc.tile_pool(name="ps", bufs=CONFIG["psbufs"], space="PSUM") as pspool,
        ):
            # W loads on the ACT HWDGE ring (parallel with x on the SP ring).
            # Ring FIFO order matters: W fc0 first, then the second half of
            # x's first chunk (its first half rides the SP ring), then the
            # remaining W chunks — so the first psum's data all lands early.
            wts = []
            for fc in range(NFC):
                wtf = wpool.tile([KC, NKC, 2 * FC], f32r, tag=f"w{fc}")
                wts.append(wtf)

            def load_w(fc):
                nc.scalar.dma_start(
                    wts[fc][:], wT[:, fc, :].rearrange("(k p) e -> p k e", p=KC)
                )

            # Chunk widths: narrow at the start (softens the DMA ramp while W
            # streams in) and at the end (shortens the serial tail chain).
            widths = [512, 512, 512, 512, 512, 512, 512, 512]
            assert sum(widths) == S

            # Prefetch chunk 0 split across both rings, then chunk 1's second
            # half on the ACT ring between the remaining W loads.
            load_w(0)
            xt0 = xpool.tile([KC, NKC, widths[0]], f32r, tag="xt")
            xT_r0 = xT[:, 0:widths[0]].rearrange("(k p) s -> p k s", p=KC)
            # scalar ring: alternate x0 k-chunks with the remaining W chunks so
            # the first psum's x arrives early while W keeps streaming
            nc.scalar.dma_start(xt0[:, 4, :], xT_r0[:, 4, :])
            nc.scalar.dma_start(xt0[:, 5, :], xT_r0[:, 5, :])
            load_w(1)
            nc.scalar.dma_start(xt0[:, 6, :], xT_r0[:, 6, :])
            nc.scalar.dma_start(xt0[:, 7, :], xT_r0[:, 7, :])
            for k in range(NKC // 2):
                nc.sync.dma_start(xt0[:, k, :], xT_r0[:, k, :])
            load_w(2)
            load_w(3)

            hprev = [None] * NFC
            off = 0
            for sc, width in enumerate(widths):
                if sc == 0:
                    xt = xt0
                else:
                    xt = xpool.tile([KC, NKC, width], f32r, tag="xt")
                    xT_r = xT[:, off:off + width].rearrange("(k p) s -> p k s", p=KC)
                    if CONFIG["xsplit"]:
                        for k in range(NKC):
                            nc.sync.dma_start(xt[:, k, :], xT_r[:, k, :])
                    else:
                        nc.sync.dma_start(xt[:], xT_r)
                for fc in range(NFC):
                    ph = pspool.tile([FC, width], f32, tag="ph")
                    pg = pspool.tile([FC, width], f32, tag="pg")
                    if CONFIG["mm_interleave"]:
                        for k in range(NKC):
                            nc.tensor.matmul(
                                ph[:], wts[fc][:, k, 0:FC], xt[:, k, :],
                                start=(k == 0), stop=(k == NKC - 1),
                            )
                            nc.tensor.matmul(
                                pg[:], wts[fc][:, k, FC:2 * FC], xt[:, k, :],
                                start=(k == 0), stop=(k == NKC - 1),
                            )
                    else:
                        for k in range(NKC):
                            nc.tensor.matmul(
                                ph[:], wts[fc][:, k, 0:FC], xt[:, k, :],
                                start=(k == 0), stop=(k == NKC - 1),
                            )
                        for k in range(NKC):
                            nc.tensor.matmul(
                                pg[:], wts[fc][:, k, FC:2 * FC], xt[:, k, :],
                                start=(k == 0), stop=(k == NKC - 1),
                            )
                    zt = epool.tile([FC, width], f32, tag="z")
                    ct = epool.tile([FC, width], f32, tag="c")
                    st = epool.tile([FC, width], f32, tag="s")
                    gt = epool.tile([FC, width], f32, tag="g")
                    ut = epool.tile([FC, width], f32, tag="u")
                    # s first: it heads the DVE critical chain (s->g->u->scan)
                    nc.scalar.activation(st[:], ph[:], AF.Sigmoid)
                    nc.scalar.activation(zt[:], pg[:], AF.Sigmoid)
                    nc.scalar.activation(ct[:], pg[:], AF.Sigmoid, scale=-1.0)
                    # g = (hidden + 0.5) max sigmoid(hidden)
                    nc.vector.scalar_tensor_tensor(
                        gt[:], ph[:], 0.5, st[:], op0=OP.add, op1=OP.max
                    )
                    ueng = nc.gpsimd if CONFIG["u_on_gpsimd"] else nc.vector
                    ueng.tensor_mul(ut[:], zt[:], gt[:])
                    ht = hpool.tile([FC, width], f32, tag=f"h{fc}")
                    pw = widths[sc - 1]
                    init = 0.0 if sc == 0 else hprev[fc][:, pw - 1:pw]
                    if CONFIG["split_last_scan"] and sc == len(widths) - 1:
                        hw_ = width // 2
                        nc.vector.tensor_tensor_scan(
                            ht[:, 0:hw_], ct[:, 0:hw_], ut[:, 0:hw_], init,
                            op0=OP.mult, op1=OP.add,
                        )
                        nc.sync.dma_start(
                            outT[fc * FC:(fc + 1) * FC, off:off + hw_], ht[:, 0:hw_]
                        )
                        nc.vector.tensor_tensor_scan(
                            ht[:, hw_:width], ct[:, hw_:width], ut[:, hw_:width],
                            ht[:, hw_ - 1:hw_], op0=OP.mult, op1=OP.add,
                        )
                        nc.sync.dma_start(
                            outT[fc * FC:(fc + 1) * FC, off + hw_:off + width],
                            ht[:, hw_:width],
                        )
                        hprev[fc] = ht
                    else:
                        nc.vector.tensor_tensor_scan(
                            ht[:], ct[:], ut[:], init, op0=OP.mult, op1=OP.add
                        )
                        hprev[fc] = ht
                        nc.sync.dma_start(
                            outT[fc * FC:(fc + 1) * FC, off:off + width], ht[:]
                        )
                off += width

    nc.compile()
    return nc


def _prep_in_maps(x: np.ndarray, W_hg: np.ndarray):
    x = np.asarray(x, dtype=np.float32)
    W_hg = np.asarray(W_hg, dtype=np.float32)
    xTs = [_round_fp32r(np.ascontiguousarray(x[b].T)) for b in range(B)]
    wTs = []
    for c in range(2):
        # [D, NFC, 2*FC]: per fc, 128 hidden cols then 128 gate cols
        wt = np.empty((D, NFC, 2 * FC), dtype=np.float32)
        for fc in range(NFC):
            rows_h = W_hg[c * DH + fc * FC:c * DH + (fc + 1) * FC]      # [FC, D]
            rows_g = W_hg[D + c * DH + fc * FC:D + c * DH + (fc + 1) * FC]
            wt[:, fc, 0:FC] = rows_h.T
            wt[:, fc, FC:2 * FC] = rows_g.T
        wTs.append(_round_fp32r(wt))
    return [{"xT": xTs[core // 2], "wT": wTs[core % 2]} for core in range(N_CORES)]


def _get_runner():
    """Build the Bass module once and cache a compiled jax callable for it.

    Mirrors bass2jax.run_bass_via_pjrt's multi-core path, but keeps the
    jitted/sharded executable so repeat kernel() calls skip re-tracing.
    """
    if "runner" in _CACHE:
        return _CACHE["runner"]

    import jax
    from jax.experimental.shard_map import shard_map
    from jax.sharding import Mesh, PartitionSpec
    from concourse import bass2jax

    if "nc" not in _CACHE:
        _CACHE["nc"] = _build()
    nc = _CACHE["nc"]
    bass2jax.install_neuronx_cc_hook()

    in_names = ["xT", "wT"]
    out_name = "outT"
    out_shape, out_dtype = (DH, S), np.float32
    partition_name = nc.partition_id_tensor.name if nc.partition_id_tensor else None

    def _body(xT, wT, zout):
        operands = [xT, wT, zout]
        if partition_name is not None:
            operands.append(bass2jax.partition_id_tensor())
        outs = bass2jax._bass_exec_p.bind(
            *operands,
            out_avals=(jax.core.ShapedArray(out_shape, out_dtype),),
            in_names=tuple(in_names + [out_name] + ([partition_name] if partition_name else [])),
            out_names=(out_name,),
            lowering_input_output_aliases=(),
            sim_require_finite=True,
            sim_require_nnan=True,
            nc=nc,
        )
        return tuple(outs)

    devices = jax.devices()[:N_CORES]
    mesh = Mesh(np.asarray(devices), ("core",))
    sharded = jax.jit(
        shard_map(
            _body, mesh=mesh,
            in_specs=(PartitionSpec("core"),) * 3,
            out_specs=(PartitionSpec("core"),),
            check_rep=False,
        ),
        donate_argnums=(2,),
        keep_unused=True,
    )

    def run(in_maps):
        concat_x = np.concatenate([m["xT"] for m in in_maps], axis=0)
        concat_w = np.concatenate([m["wT"] for m in in_maps], axis=0)
        zeros = np.zeros((N_CORES * DH, S), np.float32)
        (out_arr,) = sharded(concat_x, concat_w, zeros)
        return np.asarray(out_arr).reshape(N_CORES, DH, S)

    _CACHE["runner"] = run
    return run


def kernel(x: np.ndarray, W_hg: np.ndarray) -> np.ndarray:
    run = _get_runner()
    in_maps = _prep_in_maps(x, W_hg)
    outs = run(in_maps)

    out = np.empty((B, S, D), dtype=np.float32)
    for core in range(N_CORES):
        b, c = core // 2, core % 2
        out[b, :, c * DH:(c + 1) * DH] = outs[core].T
    return out



# revision 2
# speedup vs baseline: 1.0216x; 1.0216x over previous
"""MinGRU forward on 8 TRN2 NeuronCores.

Math (linear-space reformulation of the reference's log-space Heinsen scan):
    hg = x @ W_hg.T                       # [B,S,2D]
    hidden, gate = split(hg)
    z = sigmoid(gate)
    c = sigmoid(-gate)                    # = 1 - z = exp(-softplus(gate))
    g = max(hidden + 0.5, sigmoid(hidden))  # == where(h>=0, h+0.5, sigmoid(h)) exactly
    u = z * g
    h[t] = c[t] * h[t-1] + u[t]           # convex combination -> bounded, stable
    out = h

The recurrence maps directly onto the DVE `tensor_tensor_scan` instruction
(state = data0*state + data1 along the free dim, fp32 internal state).

Sharding: 8 cores = 4 batches x 2 feature-halves (512 features each).
No cross-core communication: the scan is per-feature independent.
Host pre-transposes x (-> xT [D,S]) and W (-> wT [D, 2*512]) so the kernel
needs no on-chip transposes.  Inputs are bf16 (halves DMA traffic and
enables fast weight load); matmul accumulates fp32 in PSUM; everything
downstream of PSUM is fp32.
"""

import numpy as np

B, S, D = 4, 4096, 1024
DH = D // 2          # features per core
N_CORES = 8
KC = 128             # contraction chunk
NKC = D // KC        # 8 k chunks
FC = 128             # feature chunk (psum partitions)
NFC = DH // FC       # 4 feature chunks

_CACHE = {}

# build-time knobs (A/B tuning)
CONFIG = {
    "in_dtype": "bf16",   # "bf16" | "f32r"
    "out_ring": "scalar",  # "scalar" | "sync"
    "widths": [512, 512, 512, 512, 512, 512, 512, 384, 128],
    "xbufs": 3,
    "psbufs": 4,
    "ebufs": 3,
    "u_on_gpsimd": False,  # compute u = z*g on GpSimd instead of DVE
    "split_last_scan": True,  # last chunk: 2 chained half-scans so out-DMA overlaps
}


def _round_fp32r(a: np.ndarray) -> np.ndarray:
    """Round fp32 array to fp32r (11 explicit mantissa bits) with RNE."""
    u = np.ascontiguousarray(a, dtype=np.float32).view(np.uint32)
    r = (u + np.uint32(0x7FF) + ((u >> np.uint32(12)) & np.uint32(1))) & np.uint32(0xFFFFF000)
    return r.view(np.float32)


def _build():
    import concourse.bacc as bacc
    import concourse.tile as tile
    import concourse.mybir as mybir

    f32 = mybir.dt.float32
    in_dt = mybir.dt.bfloat16 if CONFIG["in_dtype"] == "bf16" else mybir.dt.float32r
    AF = mybir.ActivationFunctionType
    OP = mybir.AluOpType

    nc = bacc.Bacc("TRN2")
    xT = nc.dram_tensor("xT", [D, S], in_dt, kind="ExternalInput")
    # wT layout: [D, NFC, 2*FC] — per feature-chunk fc, 128 hidden cols then
    # 128 gate cols, contiguous, so each fc's weights are one DMA.
    wT = nc.dram_tensor("wT", [D, NFC, 2 * FC], in_dt, kind="ExternalInput")
    outT = nc.dram_tensor("outT", [DH, S], f32, kind="ExternalOutput")

    widths = CONFIG["widths"]
    assert sum(widths) == S

    with tile.TileContext(nc) as tc:
        with (
            tc.tile_pool(name="w", bufs=1) as wpool,
            tc.tile_pool(name="x", bufs=CONFIG["xbufs"]) as xpool,
            tc.tile_pool(name="ew", bufs=CONFIG["ebufs"]) as epool,
            tc.tile_pool(name="h", bufs=2) as hpool,
            tc.tile_pool(name="ps", bufs=CONFIG["psbufs"], space="PSUM") as pspool,
        ):
            # W rides the ACT HWDGE ring; x rides the SP ring.  The ramp is
            # DMA-critical: the very first descriptors on each ring are the
            # small slices the first matmuls need, so the PE starts within
            # ~1us and stays busy (keeping HAM's clock-gate warming early).
            wts = []
            for fc in range(NFC):
                wtf = wpool.tile([KC, NKC, 2 * FC], in_dt, tag=f"w{fc}")
                wts.append(wtf)

            wT_r = [wT[:, fc, :].rearrange("(k p) e -> p k e", p=KC) for fc in range(NFC)]
            # fc0 weights split per k-pair so w[k0] lands first.
            for k0 in range(0, NKC, 2):
                nc.scalar.dma_start(wts[0][:, k0:k0 + 2, :], wT_r[0][:, k0:k0 + 2, :])
            xt0 = xpool.tile([KC, NKC, widths[0]], in_dt, tag="xt")
            xT_r0 = xT[:, 0:widths[0]].rearrange("(k p) s -> p k s", p=KC)
            for k in range(NKC):
                nc.sync.dma_start(xt0[:, k, :], xT_r0[:, k, :])
            for fc in range(1, NFC):
                nc.scalar.dma_start(wts[fc][:], wT_r[fc])

            out_eng = nc.scalar if CONFIG["out_ring"] == "scalar" else nc.sync

            hprev = [None] * NFC
            off = 0
            for sc, width in enumerate(widths):
                if sc == 0:
                    xt = xt0
                else:
                    xt = xpool.tile([KC, NKC, width], in_dt, tag="xt")
                    xT_r = xT[:, off:off + width].rearrange("(k p) s -> p k s", p=KC)
                    if sc == 1:
                        # still inside the ramp: split so data streams in early
                        nc.sync.dma_start(xt[:, 0:4, :], xT_r[:, 0:4, :])
                        nc.sync.dma_start(xt[:, 4:8, :], xT_r[:, 4:8, :])
                    else:
                        nc.sync.dma_start(xt[:], xT_r)
                for fc in range(NFC):
                    ph = pspool.tile([FC, width], f32, tag="ph")
                    pg = pspool.tile([FC, width], f32, tag="pg")
                    for k in range(NKC):
                        nc.tensor.matmul(
                            ph[:], wts[fc][:, k, 0:FC], xt[:, k, :],
                            start=(k == 0), stop=(k == NKC - 1),
                        )
                    for k in range(NKC):
                        nc.tensor.matmul(
                            pg[:], wts[fc][:, k, FC:2 * FC], xt[:, k, :],
                            start=(k == 0), stop=(k == NKC - 1),
                        )
                    zt = epool.tile([FC, width], f32, tag="z")
                    ct = epool.tile([FC, width], f32, tag="c")
                    st = epool.tile([FC, width], f32, tag="s")
                    gt = epool.tile([FC, width], f32, tag="g")
                    ut = epool.tile([FC, width], f32, tag="u")
                    # s first: it heads the DVE critical chain (s->g->u->scan)
                    nc.scalar.activation(st[:], ph[:], AF.Sigmoid)
                    nc.scalar.activation(zt[:], pg[:], AF.Sigmoid)
                    nc.scalar.activation(ct[:], pg[:], AF.Sigmoid, scale=-1.0)
                    # g = (hidden + 0.5) max sigmoid(hidden)
                    nc.vector.scalar_tensor_tensor(
                        gt[:], ph[:], 0.5, st[:], op0=OP.add, op1=OP.max
                    )
                    ueng = nc.gpsimd if CONFIG["u_on_gpsimd"] else nc.vector
                    ueng.tensor_mul(ut[:], zt[:], gt[:])
                    ht = hpool.tile([FC, width], f32, tag=f"h{fc}")
                    pw = widths[sc - 1]
                    init = 0.0 if sc == 0 else hprev[fc][:, pw - 1:pw]
                    if CONFIG["split_last_scan"] and sc == len(widths) - 1:
                        hw_ = width // 2
                        nc.vector.tensor_tensor_scan(
                            ht[:, 0:hw_], ct[:, 0:hw_], ut[:, 0:hw_], init,
                            op0=OP.mult, op1=OP.add,
                        )
                        out_eng.dma_start(
                            outT[fc * FC:(fc + 1) * FC, off:off + hw_], ht[:, 0:hw_]
                        )
                        nc.vector.tensor_tensor_scan(
                            ht[:, hw_:width], ct[:, hw_:width], ut[:, hw_:width],
                            ht[:, hw_ - 1:hw_], op0=OP.mult, op1=OP.add,
                        )
                        out_eng.dma_start(
                            outT[fc * FC:(fc + 1) * FC, off + hw_:off + width],
                            ht[:, hw_:width],
                        )
                        hprev[fc] = ht
                    else:
                        nc.vector.tensor_tensor_scan(
                            ht[:], ct[:], ut[:], init, op0=OP.mult, op1=OP.add
                        )
                        hprev[fc] = ht
                        out_eng.dma_start(
                            outT[fc * FC:(fc + 1) * FC, off:off + width], ht[:]
                        )
                off += width

    nc.compile()
    return nc


def _to_in_dtype(a: np.ndarray) -> np.ndarray:
    if CONFIG["in_dtype"] == "bf16":
        import ml_dtypes
        return np.ascontiguousarray(a, dtype=np.float32).astype(ml_dtypes.bfloat16)
    return _round_fp32r(a)


def _prep_in_maps(x: np.ndarray, W_hg: np.ndarray):
    x = np.asarray(x, dtype=np.float32)
    W_hg = np.asarray(W_hg, dtype=np.float32)
    xTs = [_to_in_dtype(np.ascontiguousarray(x[b].T)) for b in range(B)]
    wTs = []
    for c in range(2):
        # [D, NFC, 2*FC]: per fc, 128 hidden cols then 128 gate cols
        wt = np.empty((D, NFC, 2 * FC), dtype=np.float32)
        for fc in range(NFC):
            rows_h = W_hg[c * DH + fc * FC:c * DH + (fc + 1) * FC]      # [FC, D]
            rows_g = W_hg[D + c * DH + fc * FC:D + c * DH + (fc + 1) * FC]
            wt[:, fc, 0:FC] = rows_h.T
            wt[:, fc, FC:2 * FC] = rows_g.T
        wTs.append(_to_in_dtype(wt))
    return [{"xT": xTs[core // 2], "wT": wTs[core % 2]} for core in range(N_CORES)]


def _get_runner():
    """Build the Bass module once and cache a compiled jax callable for it.

    Mirrors bass2jax.run_bass_via_pjrt's multi-core path, but keeps the
    jitted/sharded executable so repeat kernel() calls skip re-tracing.
    """
    if "runner" in _CACHE:
        return _CACHE["runner"]

    import jax
    from jax.experimental.shard_map import shard_map
    from jax.sharding import Mesh, PartitionSpec
    from concourse import bass2jax

    if "nc" not in _CACHE:
        _CACHE["nc"] = _build()
    nc = _CACHE["nc"]
    bass2jax.install_neuronx_cc_hook()

    in_names = ["xT", "wT"]
    out_name = "outT"
    out_shape, out_dtype = (DH, S), np.float32
    partition_name = nc.partition_id_tensor.name if nc.partition_id_tensor else None

    def _body(xT, wT, zout):
        operands = [xT, wT, zout]
        if partition_name is not None:
            operands.append(bass2jax.partition_id_tensor())
        outs = bass2jax._bass_exec_p.bind(
            *operands,
            out_avals=(jax.core.ShapedArray(out_shape, out_dtype),),
            in_names=tuple(in_names + [out_name] + ([partition_name] if partition_name else [])),
            out_names=(out_name,),
            lowering_input_output_aliases=(),
            sim_require_finite=True,
            sim_require_nnan=True,
            nc=nc,
        )
        return tuple(outs)

    devices = jax.devices()[:N_CORES]
    mesh = Mesh(np.asarray(devices), ("core",))
    sharded = jax.jit(
        shard_map(
            _body, mesh=mesh,
            in_specs=(PartitionSpec("core"),) * 3,
            out_specs=(PartitionSpec("core"),),
            check_rep=False,
        ),
        donate_argnums=(2,),
        keep_unused=True,
    )

    def run(in_maps):
        concat_x = np.concatenate([m["xT"] for m in in_maps], axis=0)
        concat_w = np.concatenate([m["wT"] for m in in_maps], axis=0)
        zeros = np.zeros((N_CORES * DH, S), np.float32)
        (out_arr,) = sharded(concat_x, concat_w, zeros)
        return np.asarray(out_arr).reshape(N_CORES, DH, S)

    _CACHE["runner"] = run
    return run


def kernel(x: np.ndarray, W_hg: np.ndarray) -> np.ndarray:
    run = _get_runner()
    in_maps = _prep_in_maps(x, W_hg)
    outs = run(in_maps)

    out = np.empty((B, S, D), dtype=np.float32)
    for core in range(N_CORES):
        b, c = core // 2, core % 2
        out[b, :, c * DH:(c + 1) * DH] = outs[core].T
    return out
